# revision 1
# baseline (speedup 1.0000x reference)
"""TP-8 Trainium2 Bass kernel for the Llama2-style greedy-decode problem.

Single NEFF per core, SPMD over 8 cores. Megatron TP-8: qkv/gate/up
column-sharded (2 heads, FF 352 per core), wo/w_down row-sharded
(AllReduce partials), lm_head vocab-sharded (4000 cols/core).
Prefill(128) + 7 KV-cache decode steps, on-device argmax
(max_with_indices + tiny stats AllGather) and indirect-DMA embedding
gather. Weights SBUF-resident bf16 (host-cast); activations f32.
clr output = logits - mean(logits) (log_softmax centering cancels).
"""
import sys

sys.path.insert(0, "/opt/trn_rl_repo")
import contextlib  # noqa: E402
import numpy as np  # noqa: E402

import concourse.bass as bass  # noqa: E402
import concourse.mybir as mybir  # noqa: E402
import concourse.tile as tile  # noqa: E402
from concourse import bacc, bass_utils  # noqa: E402

F32 = mybir.dt.float32
F32R = mybir.dt.float32r
BF16 = mybir.dt.bfloat16
U32 = mybir.dt.uint32
AX = mybir.AxisListType
AF = mybir.ActivationFunctionType
ALU = mybir.AluOpType

NH, D, FF, NL, B, L, T_NEW, V, HOUT = 16, 1024, 2816, 2, 2, 128, 8, 32000, 1124
EPS = 1e-5
ROPE_BASE = 10000.0
TP = 8
HC = NH // TP          # 2 heads per core
HD = D // NH           # 64
QC = HC * HD           # 128 local qkv cols
FS = FF // TP          # 352
VS = V // TP           # 4000
SMAX = L + T_NEW       # 136
PT = B * L             # 256
KT = D // 128          # 8
RG = [list(range(TP))]
NVC = 8
VCW = VS // NVC        # 500


def build():
    nc = bacc.Bacc("TRN2", target_bir_lowering=False, debug=False, num_devices=TP)

    def inp(name, shape, dtype=F32):
        return nc.dram_tensor(name, shape, dtype, kind="ExternalInput")

    h0T = inp("h0T", [D, PT])
    wqkv_in = [[inp(f"w{w}{l}", [D, QC], BF16) for w in "qkv"] for l in range(NL)]
    wo_in = [inp(f"wo{l}", [QC, D], BF16) for l in range(NL)]
    wg_in = [inp(f"wg{l}", [D, FS], BF16) for l in range(NL)]
    wu_in = [inp(f"wu{l}", [D, FS], BF16) for l in range(NL)]
    wd_in = [inp(f"wd{l}", [FS, D], BF16) for l in range(NL)]
    lmh_in = inp("lmh", [D, VS], BF16)
    emb_in = inp("emb", [V, D])
    pcosT_in = inp("pcosT", [128, PT])
    psinT_in = inp("psinT", [128, PT])
    dcs_in = inp("dcs", [B, (T_NEW - 1) * 8 * HD])
    cmask_in = inp("cmask", [L, L])
    idf_in = inp("idf", [128, 128])
    idb_in = inp("idb", [128, 128], BF16)
    coreoff_in = inp("coreoff", [B, 1])
    iotav_in = inp("iotav", [B, 1000])
    selbd_in = inp("selbd", [TP * B, B])
    out_t = nc.dram_tensor("out", [B, T_NEW, HOUT], F32, kind="ExternalOutput")

    with tile.TileContext(nc) as tc:
        ctx = contextlib.ExitStack()
        with ctx:
            wp = ctx.enter_context(tc.tile_pool(name="wts", bufs=1))
            cp = ctx.enter_context(tc.tile_pool(name="const", bufs=1))
            kvp = ctx.enter_context(tc.tile_pool(name="kv", bufs=1))
            sb = ctx.enter_context(tc.tile_pool(name="work", bufs=2))
            hb = ctx.enter_context(tc.tile_pool(name="hrows", bufs=2))
            dp = ctx.enter_context(tc.tile_pool(name="dram", bufs=2, space="DRAM"))

            @contextlib.contextmanager
            def psum_pool(name, bufs=1):
                with tc.tile_pool(name=name, bufs=bufs, space="PSUM") as p:
                    yield p

            # ---- constants ----
            idf = cp.tile([128, 128], F32, tag="idf")
            nc.sync.dma_start(idf[:], idf_in[:])
            idb = cp.tile([128, 128], BF16, tag="idb")
            nc.sync.dma_start(idb[:], idb_in[:])
            ones = cp.tile([128, 1], F32, tag="ones")
            nc.vector.memset(ones[:], 1.0)
            ones_bf = cp.tile([128, 1], BF16, tag="onesbf")
            nc.vector.memset(ones_bf[:], 1.0)
            onesr = cp.tile([1, 128], F32, tag="onesr")
            nc.vector.memset(onesr[:], 1.0)
            bigc = cp.tile([1, 8], F32, tag="bigc")
            nc.vector.memset(bigc[:], 1e12)
            iotav = cp.tile([B, 1000], F32, tag="iotav")
            nc.sync.dma_start(iotav[:], iotav_in[:])
            selbd = cp.tile([TP * B, B], F32, tag="selbd")
            nc.sync.dma_start(selbd[:], selbd_in[:])
            epsc = cp.tile([128, 1], F32, tag="epsc")
            nc.vector.memset(epsc[:], EPS)
            pcosT = cp.tile([128, PT], F32, tag="pcos")
            nc.sync.dma_start(pcosT[:], pcosT_in[:])
            psinT = cp.tile([128, PT], F32, tag="psin")
            nc.sync.dma_start(psinT[:], psinT_in[:])
            cmask = cp.tile([L, L], F32, tag="cmask")
            nc.sync.dma_start(cmask[:], cmask_in[:])
            coreoff = cp.tile([B, 1], F32, tag="coff")
            nc.sync.dma_start(coreoff[:], coreoff_in[:])

            # ---- weights (bf16 from host), issued on the GpSimd queue so the
            # sync queue stays free for compute-critical DMAs ----
            wqkv_sb, wo_sb, wg_sb, wu_sb, wd_sb = [], [], [], [], []
            for l in range(NL):
                trio = []
                for wi, win in enumerate(wqkv_in[l]):
                    wsb = wp.tile([128, KT * QC], BF16, tag=f"w{wi}{l}")
                    for k in range(KT):
                        nc.gpsimd.dma_start(wsb[:, k * QC:(k + 1) * QC],
                                            win[k * 128:(k + 1) * 128, :])
                    trio.append(wsb)
                wqkv_sb.append(trio)
                wos = wp.tile([128, D], BF16, tag=f"wo{l}")
                nc.gpsimd.dma_start(wos[:], wo_in[l][:, :])
                wo_sb.append(wos)
                for name, win, lst in (("g", wg_in[l], wg_sb), ("u", wu_in[l], wu_sb)):
                    wsb = wp.tile([128, KT * FS], BF16, tag=f"w{name}s{l}")
                    for k in range(KT):
                        nc.gpsimd.dma_start(wsb[:, k * FS:(k + 1) * FS],
                                            win[k * 128:(k + 1) * 128, :])
                    lst.append(wsb)
                wsb = wp.tile([128, 3 * D], BF16, tag=f"wd{l}")
                for j in range(3):
                    rows = min(128, FS - j * 128)
                    nc.gpsimd.dma_start(wsb[:rows, j * D:(j + 1) * D],
                                        wd_in[l][j * 128:j * 128 + rows, :])
                wd_sb.append(wsb)
            lmh_sb = wp.tile([128, KT * VS], BF16, tag="lmh")
            for k in range(KT):
                nc.gpsimd.dma_start(lmh_sb[:, k * VS:(k + 1) * VS],
                                    lmh_in[k * 128:(k + 1) * 128, :])

            # ---- KV caches ----
            kT_c = [kvp.tile([128, B * SMAX], BF16, tag=f"kT{l}", name=f"kT{l}")
                    for l in range(NL)]
            v0_c = [kvp.tile([128, 4 * HD], BF16, tag=f"v0{l}", name=f"v0{l}")
                    for l in range(NL)]
            v1_c = [kvp.tile([8, 4 * HD], BF16, tag=f"v1{l}", name=f"v1{l}")
                    for l in range(NL)]

            def rsqrt_nt(m_ap, shape, tag, c=1.0):
                # 1/sqrt(m) vector-only (keeps scalar engine on one act-table
                # set): seed y0=min(1/m, 1.8) converges for m in [0.1, 10];
                # callers prescale m near 1 with a per-site constant.
                yt = sb.tile(shape, F32, tag=f"{tag}b", name=f"{tag}b")
                nc.vector.reciprocal(yt[:], m_ap)
                nc.vector.tensor_scalar(out=yt[:], in0=yt[:], scalar1=1.8,
                                        scalar2=None, op0=ALU.min)
                y = yt[:]
                tn = sb.tile(shape, F32, tag=f"{tag}t", name=f"{tag}t")
                for _ in range(5):
                    nc.vector.tensor_tensor(out=tn[:], in0=y, in1=y, op=ALU.mult)
                    nc.vector.tensor_tensor(out=tn[:], in0=tn[:], in1=m_ap,
                                            op=ALU.mult)
                    nc.vector.tensor_scalar(out=tn[:], in0=tn[:], scalar1=-0.5,
                                            scalar2=1.5, op0=ALU.mult,
                                            op1=ALU.add)
                    nc.vector.tensor_tensor(out=y, in0=y, in1=tn[:], op=ALU.mult)
                if c != 1.0:
                    nc.vector.tensor_scalar(out=y, in0=y,
                                            scalar1=float(c ** -0.5),
                                            scalar2=None, op0=ALU.mult)
                return y

            # ================= prefill =================
            hT = [hb.tile([128, PT], F32, tag=f"hT{k}", name=f"hT{k}", bufs=1)
                  for k in range(KT)]
            for k in range(KT):
                nc.sync.dma_start(hT[k][:], h0T[k * 128:(k + 1) * 128, :])

            def rms_norm_T(h_tiles, c=1.0):
                with psum_pool("pnorm", bufs=1) as pn:
                    ssp = pn.tile([1, PT], F32, tag="ssp", name="ssp")
                    for k in range(KT):
                        s = sb.tile([128, PT], F32, tag="sq", name="sq", bufs=1)
                        nc.vector.tensor_tensor(out=s[:], in0=h_tiles[k][:],
                                                in1=h_tiles[k][:], op=ALU.mult)
                        nc.tensor.matmul(out=ssp[:], lhsT=ones[:], rhs=s[:],
                                         start=(k == 0), stop=(k == KT - 1))
                    sd = sb.tile([1, PT], F32, tag="sd", name="sd")
                    nc.vector.tensor_scalar(out=sd[:], in0=ssp[:],
                                            scalar1=1.0 / (D * c),
                                            scalar2=EPS / c,
                                            op0=ALU.mult, op1=ALU.add)
                r = rsqrt_nt(sd[:], [1, PT], "rpf", c)
                xb = []
                with psum_pool("prbc", bufs=1) as pb:
                    rbc = pb.tile([128, PT], F32, tag="rbc", name="rbc")
                    nc.tensor.matmul(out=rbc[:], lhsT=onesr[:], rhs=r,
                                     start=True, stop=True)
                    for k in range(KT):
                        x = sb.tile([128, PT], BF16, tag=f"xs{k}", name=f"xs{k}",
                                    bufs=1)
                        nc.vector.tensor_tensor(out=x[:], in0=h_tiles[k][:],
                                                in1=rbc[:], op=ALU.mult)
                        xb.append(x)
                return xb

            def ropeT(psrc, cache_dst=None):
                rot = sb.tile([128, PT], F32, tag="rot", name="rot", bufs=1)
                for h in range(HC):
                    b0 = h * HD
                    nc.scalar.activation(rot[b0:b0 + 32, :], psrc[b0 + 32:b0 + 64, :],
                                         AF.Copy)
                    nc.scalar.activation(rot[b0 + 32:b0 + 64, :], psrc[b0:b0 + 32, :],
                                         AF.Copy)
                t1 = sb.tile([128, PT], F32, tag="rt1", name="rt1", bufs=1)
                nc.vector.tensor_tensor(out=t1[:], in0=psrc[:], in1=pcosT[:],
                                        op=ALU.mult)
                nc.vector.tensor_tensor(out=rot[:], in0=rot[:], in1=psinT[:],
                                        op=ALU.mult)
                if cache_dst is None:
                    o = sb.tile([128, PT], BF16, tag="qro", name="qro", bufs=1)
                    nc.vector.tensor_tensor(out=o[:], in0=t1[:], in1=rot[:],
                                            op=ALU.add)
                    return o
                for b in range(B):
                    nc.vector.tensor_tensor(
                        out=cache_dst[:, b * SMAX:b * SMAX + L],
                        in0=t1[:, b * L:(b + 1) * L],
                        in1=rot[:, b * L:(b + 1) * L], op=ALU.add)
                return None

            def qkv_matmul(pool, xb, wsb, width, tag):
                ps = pool.tile([128, PT], F32, tag=tag, name=tag)
                for k in range(KT):
                    nc.tensor.matmul(out=ps[:width, :],
                                     lhsT=wsb[:, k * width:(k + 1) * width],
                                     rhs=xb[k][:], start=(k == 0), stop=(k == KT - 1))
                return ps

            def ar_big(psum_flat, h_tiles):
                bi = dp.tile([128, KT * PT], F32, tag="abi", name="abi")
                bo = dp.tile([128, KT * PT], F32, tag="abo", name="abo")
                for k in range(KT):
                    ev = sb.tile([128, PT], F32, tag="aev", name="aev", bufs=1)
                    nc.vector.tensor_copy(ev[:], psum_flat[:, k * PT:(k + 1) * PT])
                    nc.sync.dma_start(bi[:, k * PT:(k + 1) * PT], ev[:])
                nc.gpsimd.collective_compute("AllReduce", ALU.add, replica_groups=RG,
                                             ins=[bi[:].opt()], outs=[bo[:].opt()])
                for k in range(KT):
                    g = sb.tile([128, PT], F32, tag="aev", name="agt", bufs=1)
                    nc.sync.dma_start(g[:], bo[:, k * PT:(k + 1) * PT])
                    nc.vector.tensor_tensor(out=h_tiles[k][:], in0=h_tiles[k][:],
                                            in1=g[:], op=ALU.add)

            RMSC = [[4e-4, 0.018], [0.1, 0.2]]
            for l in range(NL):
                xb = rms_norm_T(hT, RMSC[l][0])
                with psum_pool("pqkv", bufs=1) as pq_pool:
                    psq = qkv_matmul(pq_pool, xb, wqkv_sb[l][0], QC, "psq")
                    qb = ropeT(psq)
                    psk = qkv_matmul(pq_pool, xb, wqkv_sb[l][1], QC, "psk")
                    ropeT(psk, cache_dst=kT_c[l])
                    psv = qkv_matmul(pq_pool, xb, wqkv_sb[l][2], QC, "psv")
                    vb = sb.tile([128, PT], BF16, tag="vb", name="vb", bufs=1)
                    nc.scalar.activation(vb[:], psv[:], AF.Copy)
                with psum_pool("pvt", bufs=2) as pv_pool:
                    for b in range(B):
                        for h in range(HC):
                            p = 2 * b + h
                            pv = pv_pool.tile([128, HD], BF16, tag="pvT", name="pvT")
                            nc.tensor.transpose(
                                pv[:], vb[h * HD:(h + 1) * HD, b * L:(b + 1) * L],
                                idb[h * HD:(h + 1) * HD, h * HD:(h + 1) * HD])
                            nc.scalar.activation(v0_c[l][:, p * HD:(p + 1) * HD],
                                                 pv[:], AF.Copy)
                oT = sb.tile([128, PT], BF16, tag="oT", name="oT", bufs=1)
                with psum_pool("pattn", bufs=2) as pa_pool:
                    for b in range(B):
                        for h in range(HC):
                            p = 2 * b + h
                            psc = pa_pool.tile([L, L], F32, tag="psc", name="psc")
                            nc.tensor.matmul(
                                out=psc[:],
                                lhsT=qb[h * HD:(h + 1) * HD, b * L:(b + 1) * L],
                                rhs=kT_c[l][h * HD:(h + 1) * HD,
                                            b * SMAX:b * SMAX + L],
                                start=True, stop=True)
                            nc.vector.tensor_tensor(out=psc[:], in0=psc[:],
                                                    in1=cmask[:], op=ALU.add)
                            mx = sb.tile([L, 1], F32, tag="mx", name="mx")
                            nc.vector.tensor_reduce(mx[:], psc[:], AX.X, ALU.max)
                            nmx = sb.tile([L, 1], F32, tag="nmx", name="nmx")
                            nc.scalar.activation(nmx[:], mx[:], AF.Copy, scale=-0.125)
                            e = sb.tile([L, L], F32, tag="esm", name="esm", bufs=1)
                            nc.scalar.activation(e[:], psc[:], AF.Exp, bias=nmx[:],
                                                 scale=0.125)
                            ssum = sb.tile([L, 1], F32, tag="ssum", name="ssum")
                            nc.vector.tensor_reduce(ssum[:], e[:], AX.X, ALU.add)
                            rr = sb.tile([L, 1], F32, tag="rrp", name="rrp")
                            nc.vector.reciprocal(rr[:], ssum[:])
                            att = sb.tile([L, L], BF16, tag="att", name="att", bufs=1)
                            nc.vector.tensor_scalar_mul(att[:], e[:], rr[:])
                            paT = pa_pool.tile([L, L], BF16, tag="paT", name="paT")
                            nc.tensor.transpose(paT[:], att[:], idb[:L, :L])
                            attT = sb.tile([L, L], BF16, tag="attT", name="attT",
                                           bufs=1)
                            nc.scalar.activation(attT[:], paT[:], AF.Copy)
                            pov = pa_pool.tile([HD, L], F32, tag="pov", name="pov")
                            nc.tensor.matmul(out=pov[:],
                                             lhsT=v0_c[l][:, p * HD:(p + 1) * HD],
                                             rhs=attT[:], start=True, stop=True)
                            nc.scalar.activation(
                                oT[h * HD:(h + 1) * HD, b * L:(b + 1) * L],
                                pov[:], AF.Copy)
                with psum_pool("pbigp", bufs=1) as pb_pool:
                    pwo = pb_pool.tile([128, KT * PT], F32, tag="pbig", name="pwo")
                    for m in range(KT):
                        nc.tensor.matmul(out=pwo[:, m * PT:(m + 1) * PT],
                                         lhsT=wo_sb[l][:, m * 128:(m + 1) * 128],
                                         rhs=oT[:], start=True, stop=True)
                    ar_big(pwo, hT)
                # --- FFN ---
                xb2 = rms_norm_T(hT, RMSC[l][1])
                af = sb.tile([128, 3 * PT], BF16, tag="af", name="af", bufs=1)
                with psum_pool("pffn", bufs=1) as pf_pool:
                    pg = pf_pool.tile([128, 3 * PT], F32, tag="pgu0", name="pg")
                    pu = pf_pool.tile([128, 3 * PT], F32, tag="pgu1", name="pu")
                    for ps, wsb in ((pg, wg_sb[l]), (pu, wu_sb[l])):
                        for j in range(3):
                            rows = min(128, FS - j * 128)
                            for k in range(KT):
                                nc.tensor.matmul(
                                    out=ps[:rows, j * PT:(j + 1) * PT],
                                    lhsT=wsb[:, k * FS + j * 128:
                                             k * FS + j * 128 + rows],
                                    rhs=xb2[k][:], start=(k == 0),
                                    stop=(k == KT - 1))
                    gs = sb.tile([128, 3 * PT], F32, tag="gsf", name="gsf", bufs=1)
                    for j in range(3):
                        rows = min(128, FS - j * 128)
                        blk = slice(j * PT, (j + 1) * PT)
                        # silu via exp only (scalar engine stays on one table)
                        nc.scalar.activation(gs[:rows, blk], pg[:rows, blk],
                                             AF.Exp, scale=-1.0)
                        nc.vector.tensor_scalar(out=gs[:rows, blk],
                                                in0=gs[:rows, blk], scalar1=1.0,
                                                scalar2=None, op0=ALU.add)
                        nc.vector.reciprocal(gs[:rows, blk], gs[:rows, blk])
                        nc.vector.tensor_tensor(out=gs[:rows, blk],
                                                in0=gs[:rows, blk],
                                                in1=pg[:rows, blk], op=ALU.mult)
                        nc.vector.tensor_tensor(out=af[:rows, blk],
                                                in0=gs[:rows, blk],
                                                in1=pu[:rows, blk], op=ALU.mult)
                with psum_pool("pbigp", bufs=1) as pb_pool:
                    pd = pb_pool.tile([128, KT * PT], F32, tag="pbig", name="pdd")
                    for m in range(KT):
                        for j in range(3):
                            rows = min(128, FS - j * 128)
                            nc.tensor.matmul(
                                out=pd[:, m * PT:(m + 1) * PT],
                                lhsT=wd_sb[l][:rows,
                                              j * D + m * 128:j * D + (m + 1) * 128],
                                rhs=af[:rows, j * PT:(j + 1) * PT],
                                start=(j == 0), stop=(j == 2))
                    ar_big(pd, hT)

            # last-token hidden -> row form [B, D]
            hrow0 = hb.tile([B, D], F32, tag="hrow", name="hrow0")
            with psum_pool("pxtp", bufs=2) as px_pool:
                for k in range(KT):
                    hl = sb.tile([128, B], F32, tag="hl", name="hl")
                    for b in range(B):
                        nc.scalar.activation(hl[:, b:b + 1],
                                             hT[k][:, b * L + L - 1:b * L + L],
                                             AF.Copy)
                    pt_ = px_pool.tile([B, 128], F32, tag="pxt", name="pxt")
                    nc.tensor.transpose(pt_[:], hl[:], idf[:, :])
                    nc.vector.tensor_copy(hrow0[:, k * 128:(k + 1) * 128], pt_[:])

            # ============ row-form helpers ============
            def rms_row(h, c=0.25):
                s = sb.tile([B, D], F32, tag="rs", name="rs", bufs=1)
                nc.vector.tensor_tensor(out=s[:], in0=h[:], in1=h[:], op=ALU.mult)
                ms = sb.tile([B, 1], F32, tag="rm", name="rm")
                nc.vector.tensor_reduce(ms[:], s[:], AX.X, ALU.add)
                sd = sb.tile([B, 1], F32, tag="rdd", name="rdd")
                nc.scalar.activation(sd[:], ms[:], AF.Sqrt, bias=epsc[:B, :],
                                     scale=1.0 / D)
                rr_ = sb.tile([B, 1], F32, tag="rr2", name="rr2")
                nc.vector.reciprocal(rr_[:], sd[:])
                r = rr_[:]
                x = sb.tile([B, D], BF16, tag="rx", name="rx")
                nc.vector.tensor_scalar_mul(x[:], h[:], r)
                return x

            def transpose_row(xrow, ncols, tag, in_f32=False):
                nt = (ncols + 127) // 128
                xT = sb.tile([128, 2 * nt], BF16, tag=tag, name=tag)
                with psum_pool("ptrp", bufs=2) as pr_pool:
                    for k in range(nt):
                        w = min(128, ncols - k * 128)
                        ptr = pr_pool.tile([128, B], F32 if in_f32 else BF16,
                                           tag="ptr", name="ptr")
                        ident = idf if in_f32 else idb
                        nc.tensor.transpose(ptr[:w, :], xrow[:, k * 128:k * 128 + w],
                                            ident[:B, :B])
                        nc.scalar.activation(xT[:w, 2 * k:2 * k + 2], ptr[:w, :],
                                             AF.Copy)
                return xT

            def ar_site(partial_sb, hin, tag, hname):
                # AllGather (floor ~4.6us vs AllReduce ~9.7us); the 16 gathered
                # rows land on 16 partitions and are summed per-b by a matmul
                # with the 0/1 selector selbd, fused with the residual add.
                bi = dp.tile([B, D], F32, tag=f"{tag}i", name=f"{tag}i")
                bo = dp.tile([TP * B, D], F32, tag=f"{tag}o", name=f"{tag}o")
                nc.sync.dma_start(bi[:], partial_sb[:])
                nc.gpsimd.collective_compute("AllGather", ALU.bypass,
                                             replica_groups=RG,
                                             ins=[bi[:].opt()], outs=[bo[:].opt()])
                g2 = sb.tile([TP * B, D], F32, tag="ag2", name=f"{tag}g2", bufs=1)
                nc.sync.dma_start(g2[:], bo[:])
                h2 = hb.tile([B, D], F32, tag="hrow", name=hname)
                with psum_pool("pags", bufs=1) as ps_pool:
                    pg2 = ps_pool.tile([B, D], F32, tag="pag", name="pag")
                    for half in range(2):
                        nc.tensor.matmul(out=pg2[:, half * 512:(half + 1) * 512],
                                         lhsT=selbd[:],
                                         rhs=g2[:, half * 512:(half + 1) * 512],
                                         start=True, stop=True)
                    nc.vector.tensor_tensor(out=h2[:], in0=hin[:], in1=pg2[:],
                                            op=ALU.add)
                return h2

            def vocab_step(tt, hrow):
                xf = rms_row(hrow)
                hfT = transpose_row(xf, D, "hfT")
                ll = sb.tile([B, VS], F32, tag="ll", name="ll", bufs=1)
                lsum = sb.tile([B, 1], F32, tag="lsum", name="lsum")
                clrs = sb.tile([B, HOUT], F32, tag="dpart", name="clrs", bufs=1)
                m8 = sb.tile([B, NVC], F32, tag="m8", name="m8")
                s8 = sb.tile([B, NVC], F32, tag="s8", name="s8")
                with psum_pool("plmp", bufs=4) as pl_pool:
                    for nt in range(NVC):
                        pl = pl_pool.tile([B, VCW], F32, tag="plm", name="plm")
                        for k in range(KT):
                            nc.tensor.matmul(
                                out=pl[:], lhsT=hfT[:, 2 * k:2 * k + 2],
                                rhs=lmh_sb[:, k * VS + nt * VCW:
                                           k * VS + (nt + 1) * VCW],
                                start=(k == 0), stop=(k == KT - 1))
                        # copy on scalar; chunk max+sum on vector — both
                        # overlap the remaining lm_head matmuls
                        nc.scalar.activation(ll[:, nt * VCW:(nt + 1) * VCW],
                                             pl[:], AF.Copy)
                        nc.vector.tensor_reduce(m8[:, nt:nt + 1], pl[:],
                                                AX.X, ALU.max)
                        nc.vector.tensor_reduce(s8[:, nt:nt + 1], pl[:],
                                                AX.X, ALU.add)
                fmx = sb.tile([B, 1], F32, tag="fmx", name="fmx")
                nc.vector.tensor_reduce(fmx[:], m8[:], AX.X, ALU.max)
                nc.vector.tensor_reduce(lsum[:], s8[:], AX.X, ALU.add)
                # argmax: min over (fmx-ll)*1e12 + iota (zero only at the max)
                lvidx = sb.tile([B, 1], F32, tag="lvidx", name="lvidx")
                with psum_pool("pcnd", bufs=1) as pc_pool:
                    cnd = pc_pool.tile([B, VS], F32, tag="cndv", name="cndv")
                    nc.vector.tensor_scalar(out=cnd[:], in0=ll[:],
                                            scalar1=fmx[:, 0:1], scalar2=-1e12,
                                            op0=ALU.subtract, op1=ALU.mult)
                    for c in range(4):
                        cs = slice(c * 1000, (c + 1) * 1000)
                        nc.vector.scalar_tensor_tensor(
                            out=cnd[:, cs], in0=cnd[:, cs],
                            scalar=float(1000 * c), in1=iotav[:],
                            op0=ALU.add, op1=ALU.add)
                    nc.vector.tensor_reduce(lvidx[:], cnd[:], AX.X, ALU.min)
                st4 = sb.tile([B, 4], F32, tag="st4", name="st4")
                nc.vector.tensor_copy(st4[:, 0:1], fmx[:, 0:1])
                nc.vector.tensor_tensor(out=st4[:, 1:2], in0=lvidx[:],
                                        in1=coreoff[:], op=ALU.add)
                nc.vector.tensor_copy(st4[:, 2:3], lsum[:])
                nc.vector.memset(st4[:, 3:4], 0.0)
                sbi = dp.tile([1, 8], F32, tag="sti", name="sti")
                sbo = dp.tile([8, 8], F32, tag="sto", name="sto")
                nc.sync.dma_start(sbi[:], st4[:])
                nc.gpsimd.collective_compute("AllGather", ALU.bypass,
                                             replica_groups=RG,
                                             ins=[sbi[:].opt()], outs=[sbo[:].opt()])
                gsr = sb.tile([1, 64], F32, tag="gsr", name="gsr")
                nc.sync.dma_start(gsr[:], sbo[:])
                # stat rows per b: maxes / global idxs / sums across cores
                rowm = sb.tile([1, 48], F32, tag="rowm", name="rowm")
                gv = gsr[0:1, :].rearrange("p (c s) -> p c s", s=8)
                for b in range(B):
                    for j in range(3):
                        nc.scalar.activation(rowm[0:1, (3 * b + j) * 8:
                                                  (3 * b + j + 1) * 8],
                                             gv[0:1, :, 4 * b + j:4 * b + j + 1],
                                             AF.Copy)
                nv = sb.tile([1, 4], F32, tag="nv", name="nv")
                mkb = sb.tile([1, 8], U32, tag="mkb", name="mkb")
                cand = sb.tile([1, 8], F32, tag="cand", name="cand")
                gmx = sb.tile([1, 1], F32, tag="gmx", name="gmx")
                for b in range(B):
                    mrow = rowm[0:1, (3 * b) * 8:(3 * b + 1) * 8]
                    irow = rowm[0:1, (3 * b + 1) * 8:(3 * b + 2) * 8]
                    srow = rowm[0:1, (3 * b + 2) * 8:(3 * b + 3) * 8]
                    nc.vector.tensor_reduce(gmx[:], mrow, AX.X, ALU.max)
                    nc.vector.tensor_scalar(out=mkb[:], in0=mrow, scalar1=gmx[:],
                                            scalar2=None, op0=ALU.is_equal)
                    nc.vector.select(cand[:], mkb[:], irow, bigc[:])
                    nc.vector.tensor_reduce(nv[0:1, b:b + 1], cand[:], AX.X, ALU.min)
                    nc.vector.tensor_reduce(nv[0:1, 2 + b:3 + b], srow, AX.X, ALU.add)
                with psum_pool("pgtp", bufs=1) as pg_pool:
                    pnid = pg_pool.tile([B, 1], F32, tag="pnid", name="pnid")
                    nc.tensor.transpose(pnid[:], nv[0:1, 0:2], idf[0:1, 0:1])
                    nidmean = sb.tile([B, 1], F32, tag="nidmean", name="nidmean")
                    nc.vector.tensor_copy(nidmean[:], pnid[:])
                    pmean = pg_pool.tile([B, 1], F32, tag="pmean", name="pmean")
                    nc.tensor.transpose(pmean[:], nv[0:1, 2:4], idf[0:1, 0:1])
                    mean2 = sb.tile([B, 1], F32, tag="mean2", name="mean2")
                    nc.scalar.activation(mean2[:], pmean[:], AF.Copy, scale=1.0 / V)
                nc.vector.tensor_scalar(out=clrs[:], in0=ll[:, 0:HOUT],
                                        scalar1=mean2[:], scalar2=None,
                                        op0=ALU.subtract)
                nc.sync.dma_start(out_t[0:B, tt:tt + 1, :], clrs[:])
                if tt == T_NEW - 1:
                    return None
                nidu = sb.tile([B, 1], U32, tag="nidu", name="nidu")
                nc.vector.tensor_copy(nidu[:], nidmean[:])
                embrow = hb.tile([B, D], F32, tag="hrow", name="embrow")
                nc.gpsimd.indirect_dma_start(
                    out=embrow[:], out_offset=None, in_=emb_in[:, :],
                    in_offset=bass.IndirectOffsetOnAxis(ap=nidu[:, 0:1], axis=0))
                return embrow

            # ============ decode ============
            DRC = [[4e-4, 0.1], [0.15, 0.25]]

            def decode_layer(l, hin, t, dcst):
                slen = L + t
                x = rms_row(hin, DRC[l][0])
                xT = transpose_row(x, D, "xT")
                qk2 = sb.tile([B, 2 * QC], F32, tag="qk2", name="qk2", bufs=1)
                vrow = sb.tile([B, QC], BF16, tag="vrow", name="vrow", bufs=1)
                with psum_pool("pdq", bufs=1) as pq_pool:
                    pq = pq_pool.tile([B, 3 * QC], F32, tag="dqkv", name="dqkv")
                    for wi in range(3):
                        for k in range(KT):
                            nc.tensor.matmul(
                                out=pq[:, wi * QC:(wi + 1) * QC],
                                lhsT=xT[:, 2 * k:2 * k + 2],
                                rhs=wqkv_sb[l][wi][:, k * QC:(k + 1) * QC],
                                start=(k == 0), stop=(k == KT - 1))
                    rot = sb.tile([B, 2 * QC], F32, tag="rot", name="drot", bufs=1)
                    for blk in range(4):
                        c0 = blk * HD
                        nc.scalar.activation(rot[:, c0:c0 + 32],
                                             pq[:, c0 + 32:c0 + 64], AF.Copy)
                        nc.scalar.activation(rot[:, c0 + 32:c0 + 64],
                                             pq[:, c0:c0 + 32], AF.Copy)
                    t1 = sb.tile([B, 2 * QC], F32, tag="rt1", name="dt1", bufs=1)
                    nc.vector.tensor_tensor(
                        out=t1[:], in0=pq[:, 0:2 * QC],
                        in1=dcst[:, 0:4 * HD], op=ALU.mult)
                    nc.vector.tensor_tensor(
                        out=rot[:], in0=rot[:],
                        in1=dcst[:, 4 * HD:8 * HD], op=ALU.mult)
                    nc.vector.tensor_tensor(out=qk2[:], in0=t1[:], in1=rot[:],
                                            op=ALU.add)
                    nc.vector.tensor_copy(vrow[:], pq[:, 2 * QC:3 * QC])
                qkT = transpose_row(qk2, 2 * QC, "qkT", in_f32=True)
                for b in range(B):
                    nc.scalar.activation(
                        kT_c[l][:, b * SMAX + slen - 1:b * SMAX + slen],
                        qkT[:, 2 + b:3 + b], AF.Copy)
                    for h in range(HC):
                        p = 2 * b + h
                        nc.sync.dma_start(
                            v1_c[l][t - 1:t, p * HD:(p + 1) * HD],
                            vrow[b:b + 1, h * HD:(h + 1) * HD])
                ao = sb.tile([B, D], F32, tag="dpart", name="dao", bufs=1)
                with psum_pool("pdap", bufs=1) as pa_pool:
                    psc = pa_pool.tile([1, 4 * 512], F32, tag="dsc", name="dsc")
                    for b in range(B):
                        for h in range(HC):
                            p = 2 * b + h
                            nc.tensor.matmul(
                                out=psc[0:1, p * 512:p * 512 + slen],
                                lhsT=qkT[h * HD:(h + 1) * HD, b:b + 1],
                                rhs=kT_c[l][h * HD:(h + 1) * HD,
                                            b * SMAX:b * SMAX + slen],
                                start=True, stop=True)
                    mxr = sb.tile([1, 4], F32, tag="dmx", name="dmx")
                    for p in range(4):
                        nc.vector.tensor_reduce(
                            mxr[0:1, p:p + 1],
                            psc[0:1, p * 512:p * 512 + slen], AX.X, ALU.max)
                    nmx = sb.tile([1, 4], F32, tag="dnmx", name="dnmx")
                    nc.scalar.activation(nmx[:], mxr[:], AF.Copy, scale=-0.125)
                    er = sb.tile([1, 4 * SMAX], F32, tag="dscr", name="dscr", bufs=1)
                    for p in range(4):
                        nc.scalar.activation(
                            er[0:1, p * SMAX:p * SMAX + slen],
                            psc[0:1, p * 512:p * 512 + slen], AF.Exp,
                            bias=nmx[0:1, p:p + 1], scale=0.125)
                    ssum = sb.tile([1, 4], F32, tag="dss", name="dss")
                    for p in range(4):
                        nc.vector.tensor_reduce(
                            ssum[0:1, p:p + 1],
                            er[0:1, p * SMAX:p * SMAX + slen], AX.X, ALU.add)
                    rr = sb.tile([1, 4], F32, tag="drr", name="drr")
                    nc.vector.reciprocal(rr[:], ssum[:])
                    for p in range(4):
                        nc.vector.tensor_scalar_mul(
                            er[0:1, p * SMAX:p * SMAX + slen],
                            er[0:1, p * SMAX:p * SMAX + slen], rr[0:1, p:p + 1])
                    paT = pa_pool.tile([L, 4], F32, tag="dpaT", name="dpaT")
                    pa1 = pa_pool.tile([8, 4], F32, tag="dpa1", name="dpa1")
                    for p in range(4):
                        nc.tensor.transpose(paT[:, p:p + 1],
                                            er[0:1, p * SMAX:p * SMAX + L],
                                            idf[0:1, 0:1])
                        nc.tensor.transpose(pa1[0:t, p:p + 1],
                                            er[0:1, p * SMAX + L:p * SMAX + slen],
                                            idf[0:1, 0:1])
                    attT = sb.tile([L, 4], BF16, tag="dattT", name="dattT")
                    nc.scalar.activation(attT[:], paT[:], AF.Copy)
                    attT1 = sb.tile([8, 4], BF16, tag="dattT1", name="dattT1")
                    nc.scalar.activation(attT1[0:t, :], pa1[0:t, :], AF.Copy)
                    pov = pa_pool.tile([HD, 4], F32, tag="dpov", name="dpov")
                    for b in range(B):
                        for h in range(HC):
                            p = 2 * b + h
                            nc.tensor.matmul(out=pov[:, p:p + 1],
                                             lhsT=v0_c[l][:, p * HD:(p + 1) * HD],
                                             rhs=attT[:, p:p + 1], start=True,
                                             stop=False)
                            nc.tensor.matmul(out=pov[:, p:p + 1],
                                             lhsT=v1_c[l][0:t, p * HD:(p + 1) * HD],
                                             rhs=attT1[0:t, p:p + 1], start=False,
                                             stop=True)
                    doT = sb.tile([128, B], BF16, tag="doT", name="doT")
                    for b in range(B):
                        for h in range(HC):
                            p = 2 * b + h
                            nc.scalar.activation(doT[h * HD:(h + 1) * HD, b:b + 1],
                                                 pov[:, p:p + 1], AF.Copy)
                with psum_pool("pdwo", bufs=1) as pw_pool:
                    pwo = pw_pool.tile([B, D], F32, tag="dpwo", name="dpwo")
                    for half in range(2):
                        nc.tensor.matmul(
                            out=pwo[:, half * 512:(half + 1) * 512], lhsT=doT[:],
                            rhs=wo_sb[l][:, half * 512:(half + 1) * 512],
                            start=True, stop=True)
                    nc.vector.tensor_copy(ao[:], pwo[:])
                h2 = ar_site(ao, hin, "dA", "h2")
                # FFN
                x2 = rms_row(h2, DRC[l][1])
                x2T = transpose_row(x2, D, "x2T")
                aro = sb.tile([B, FS], F32, tag="daro", name="daro", bufs=1)
                with psum_pool("pdfp", bufs=1) as pf_pool:
                    pg = pf_pool.tile([B, 2 * 512], F32, tag="dpgu", name="dpgu")
                    for gi, wsb in enumerate((wg_sb[l], wu_sb[l])):
                        for k in range(KT):
                            nc.tensor.matmul(out=pg[:, gi * 512:gi * 512 + FS],
                                             lhsT=x2T[:, 2 * k:2 * k + 2],
                                             rhs=wsb[:, k * FS:(k + 1) * FS],
                                             start=(k == 0), stop=(k == KT - 1))
                    gsl = sb.tile([B, FS], F32, tag="dgs", name="dgs", bufs=1)
                    nc.scalar.activation(gsl[:], pg[:, 0:FS], AF.Exp, scale=-1.0)
                    nc.vector.tensor_scalar(out=gsl[:], in0=gsl[:], scalar1=1.0,
                                            scalar2=None, op0=ALU.add)
                    nc.vector.reciprocal(gsl[:], gsl[:])
                    nc.vector.tensor_tensor(out=gsl[:], in0=gsl[:],
                                            in1=pg[:, 0:FS], op=ALU.mult)
                    nc.vector.tensor_tensor(out=aro[:], in0=gsl[:],
                                            in1=pg[:, 512:512 + FS], op=ALU.mult)
                aT = transpose_row(aro, FS, "daT", in_f32=True)
                fo = sb.tile([B, D], F32, tag="dpart", name="dfo", bufs=1)
                with psum_pool("pddp", bufs=1) as pd_pool:
                    pd = pd_pool.tile([B, D], F32, tag="dpd", name="dpd")
                    for half in range(2):
                        for j in range(3):
                            rows = min(128, FS - j * 128)
                            nc.tensor.matmul(
                                out=pd[:, half * 512:(half + 1) * 512],
                                lhsT=aT[:rows, 2 * j:2 * j + 2],
                                rhs=wd_sb[l][:rows, j * D + half * 512:
                                             j * D + (half + 1) * 512],
                                start=(j == 0), stop=(j == 2))
                    nc.vector.tensor_copy(fo[:], pd[:])
                h3 = ar_site(fo, h2, "dF", "h3")
                return h3

            hcur = vocab_step(0, hrow0)
            for t in range(1, T_NEW):
                dcst = sb.tile([B, 8 * HD], F32, tag="dcst", name="dcst", bufs=2)
                nc.sync.dma_start(dcst[:], dcs_in[:, (t - 1) * 8 * HD:t * 8 * HD])
                for l in range(NL):
                    hcur = decode_layer(l, hcur, t, dcst)
                hcur = vocab_step(t, hcur)

    nc.compile()
    return nc


def make_in_maps(inputs):
    import ml_dtypes
    bf = ml_dtypes.bfloat16
    ii = {k: np.asarray(v) for k, v in inputs.items()}
    embed = ii["embed"].astype(np.float32)
    tokens = ii["input_ids"].astype(np.int64)
    h0 = embed[tokens]
    h0T = np.ascontiguousarray(h0.transpose(2, 0, 1).reshape(D, PT)).astype(np.float32)

    inv = ROPE_BASE ** (-np.arange(32, dtype=np.float64) / 32)
    dd = np.arange(HD)
    sgn = np.where(dd < 32, -1.0, 1.0)
    fr = inv[dd % 32]

    pos_p = np.tile(np.arange(L), B)
    ang_p = np.outer(fr, pos_p)
    pcosT = np.tile(np.cos(ang_p), (2, 1)).astype(np.float32)
    psinT = np.tile(sgn[:, None] * np.sin(ang_p), (2, 1)).astype(np.float32)

    pos_d = np.arange(L, L + T_NEW - 1)
    ang_d = np.outer(pos_d, fr)
    dcos = np.tile(np.cos(ang_d), (1, 4))                   # [T_NEW-1, 4*HD]
    dsin = np.tile(np.sin(ang_d) * sgn[None, :], (1, 4))
    dcs = np.tile(np.concatenate([dcos, dsin], axis=1).reshape(1, -1),
                  (B, 1)).astype(np.float32)

    q_idx = np.arange(L)[:, None]
    cmask = np.where(np.arange(L)[None, :] <= q_idx, 0.0, -8e9).astype(np.float32)
    idf = np.eye(128, dtype=np.float32)
    idb = np.eye(128).astype(bf)

    an, fn, fin = ii["attn_norm"], ii["ffn_norm"], ii["final_norm"]
    in_maps = []
    for c in range(TP):
        m = {"h0T": h0T, "emb": embed, "pcosT": pcosT, "psinT": psinT,
             "dcs": dcs, "cmask": cmask, "idf": idf,
             "idb": idb, "coreoff": np.full((B, 1), c * VS, np.float32),
             "iotav": np.tile(np.arange(1000, dtype=np.float32), (B, 1)),
             "selbd": np.array([[1.0 if r % B == b else 0.0 for b in range(B)]
                                for r in range(TP * B)], np.float32)}
        for l in range(NL):
            for w, key in (("q", "wq"), ("k", "wk"), ("v", "wv")):
                m[f"w{w}{l}"] = np.ascontiguousarray(
                    (an[l][:, None] * ii[key][l])[:, c * QC:(c + 1) * QC]).astype(bf)
            m[f"wo{l}"] = np.ascontiguousarray(
                ii["wo"][l][c * QC:(c + 1) * QC, :]).astype(bf)
            m[f"wg{l}"] = np.ascontiguousarray(
                (fn[l][:, None] * ii["w_gate"][l])[:, c * FS:(c + 1) * FS]).astype(bf)
            m[f"wu{l}"] = np.ascontiguousarray(
                (fn[l][:, None] * ii["w_up"][l])[:, c * FS:(c + 1) * FS]).astype(bf)
            m[f"wd{l}"] = np.ascontiguousarray(
                ii["w_down"][l][c * FS:(c + 1) * FS, :]).astype(bf)
        m["lmh"] = np.ascontiguousarray(
            (fin[:, None] * ii["lm_head"])[:, c * VS:(c + 1) * VS]).astype(bf)
        in_maps.append(m)
    return in_maps


_NC_CACHE = {}


def kernel(**inputs):
    if "nc" not in _NC_CACHE:
        _NC_CACHE["nc"] = build()
    nc = _NC_CACHE["nc"]
    in_maps = make_in_maps(inputs)
    res = bass_utils.run_bass_kernel_spmd(nc, in_maps, core_ids=list(range(TP)))
    return np.asarray(res.results[0]["out"], dtype=np.float32)



# revision 15
# speedup vs baseline: 1.0480x; 1.0480x over previous
"""TP-8 Trainium2 Bass kernel for the Llama2-style greedy-decode problem.

Single NEFF per core, SPMD over 8 cores. Megatron TP-8: qkv/gate/up
column-sharded (2 heads, FF 352 per core), wo/w_down row-sharded
(AllReduce partials), lm_head vocab-sharded (4000 cols/core).
Prefill(128) + 7 KV-cache decode steps, on-device argmax
(max_with_indices + tiny stats AllGather) and indirect-DMA embedding
gather. Weights SBUF-resident bf16 (host-cast); activations f32.
clr output = logits - mean(logits) (log_softmax centering cancels).
"""
import sys

sys.path.insert(0, "/opt/trn_rl_repo")
import contextlib  # noqa: E402
import numpy as np  # noqa: E402

import concourse.bass as bass  # noqa: E402
import concourse.mybir as mybir  # noqa: E402
import concourse.tile as tile  # noqa: E402
from concourse import bacc, bass_utils  # noqa: E402

F32 = mybir.dt.float32
F32R = mybir.dt.float32r
BF16 = mybir.dt.bfloat16
U32 = mybir.dt.uint32
AX = mybir.AxisListType
AF = mybir.ActivationFunctionType
ALU = mybir.AluOpType

NH, D, FF, NL, B, L, T_NEW, V, HOUT = 16, 1024, 2816, 2, 2, 128, 8, 32000, 1124
EPS = 1e-5
ROPE_BASE = 10000.0
TP = 8
HC = NH // TP          # 2 heads per core
HD = D // NH           # 64
QC = HC * HD           # 128 local qkv cols
FS = FF // TP          # 352
VS = V // TP           # 4000
SMAX = L + T_NEW       # 136
PT = B * L             # 256
KT = D // 128          # 8
RG = [list(range(TP))]
NVC = 8
VCW = VS // NVC        # 500


def build():
    nc = bacc.Bacc("TRN2", target_bir_lowering=False, debug=False, num_devices=TP)

    def inp(name, shape, dtype=F32):
        return nc.dram_tensor(name, shape, dtype, kind="ExternalInput")

    h0T = inp("h0T", [D, PT])
    wqkv_in = [[inp(f"w{w}{l}", [D, QC], BF16) for w in "qkv"] for l in range(NL)]
    wo_in = [inp(f"wo{l}", [QC, D], BF16) for l in range(NL)]
    wg_in = [inp(f"wg{l}", [D, FS], BF16) for l in range(NL)]
    wu_in = [inp(f"wu{l}", [D, FS], BF16) for l in range(NL)]
    wd_in = [inp(f"wd{l}", [FS, D], BF16) for l in range(NL)]
    lmh_in = inp("lmh", [D, VS], BF16)
    wmean_in = inp("wmean", [D, 1], BF16)
    emb_in = inp("emb", [V, D])
    pcosT_in = inp("pcosT", [128, PT])
    psinT_in = inp("psinT", [128, PT])
    dcs_in = inp("dcs", [B, (T_NEW - 1) * 8 * HD])
    cmask_in = inp("cmask", [L, L])
    idf_in = inp("idf", [128, 128])
    idb_in = inp("idb", [128, 128], BF16)
    chunkbase_in = inp("chunkbase", [B, 8 * NVC])
    selbd_in = inp("selbd", [TP * B, B])
    out_t = nc.dram_tensor("out", [B, T_NEW, HOUT], F32, kind="ExternalOutput")

    with tile.TileContext(nc) as tc:
        ctx = contextlib.ExitStack()
        with ctx:
            wp = ctx.enter_context(tc.tile_pool(name="wts", bufs=1))
            cp = ctx.enter_context(tc.tile_pool(name="const", bufs=1))
            kvp = ctx.enter_context(tc.tile_pool(name="kv", bufs=1))
            sb = ctx.enter_context(tc.tile_pool(name="work", bufs=2))
            hb = ctx.enter_context(tc.tile_pool(name="hrows", bufs=2))
            dp = ctx.enter_context(tc.tile_pool(name="dram", bufs=2, space="DRAM"))

            @contextlib.contextmanager
            def psum_pool(name, bufs=1):
                with tc.tile_pool(name=name, bufs=bufs, space="PSUM") as p:
                    yield p

            # ---- constants ----
            idf = cp.tile([128, 128], F32, tag="idf")
            nc.sync.dma_start(idf[:], idf_in[:])
            idb = cp.tile([128, 128], BF16, tag="idb")
            nc.sync.dma_start(idb[:], idb_in[:])
            ones = cp.tile([128, 1], F32, tag="ones")
            nc.vector.memset(ones[:], 1.0)
            ones_bf = cp.tile([128, 1], BF16, tag="onesbf")
            nc.vector.memset(ones_bf[:], 1.0)
            onesr = cp.tile([1, 128], F32, tag="onesr")
            nc.vector.memset(onesr[:], 1.0)
            bigc = cp.tile([1, 8], F32, tag="bigc")
            nc.vector.memset(bigc[:], 1e12)
            big64 = cp.tile([B, 8 * NVC], F32, tag="big64")
            nc.vector.memset(big64[:], 1e12)
            chunkbase = cp.tile([B, 8 * NVC], F32, tag="chunkbase")
            nc.sync.dma_start(chunkbase[:], chunkbase_in[:])
            selbd = cp.tile([TP * B, B], F32, tag="selbd")
            nc.sync.dma_start(selbd[:], selbd_in[:])
            epsc = cp.tile([128, 1], F32, tag="epsc")
            nc.vector.memset(epsc[:], EPS)
            pcosT = cp.tile([128, PT], F32, tag="pcos")
            nc.sync.dma_start(pcosT[:], pcosT_in[:])
            psinT = cp.tile([128, PT], F32, tag="psin")
            nc.sync.dma_start(psinT[:], psinT_in[:])
            cmask = cp.tile([L, L], F32, tag="cmask")
            nc.sync.dma_start(cmask[:], cmask_in[:])

            # ---- weights (bf16 from host): one batched DMA per tensor,
            # spread over queues. Layer-0 qkv on sync (needed first), L0
            # ffn + wo on scalar, all of layer 1 on gpsimd, lm_head on
            # vector (biggest, needed last). ----
            VS1 = VS + 1          # +1 col for w_mean (logit-mean trick)
            wqkv_sb, wo_sb, wg_sb, wu_sb, wd_sb = [], [], [], [], []
            for l in range(NL):
                q = nc.sync if l == 0 else nc.gpsimd
                q2 = nc.scalar if l == 0 else nc.gpsimd
                trio = []
                for wi, win in enumerate(wqkv_in[l]):
                    wsb = wp.tile([128, KT * QC], BF16, tag=f"w{wi}{l}")
                    q.dma_start(wsb[:].rearrange("p (k c) -> p k c", c=QC),
                                win[:, :].rearrange("(k p) c -> p k c", p=128))
                    trio.append(wsb)
                wqkv_sb.append(trio)
                wos = wp.tile([128, D], BF16, tag=f"wo{l}")
                q2.dma_start(wos[:], wo_in[l][:, :])
                wo_sb.append(wos)
                for name, win, lst in (("g", wg_in[l], wg_sb), ("u", wu_in[l], wu_sb)):
                    wsb = wp.tile([128, KT * FS], BF16, tag=f"w{name}s{l}")
                    q2.dma_start(wsb[:].rearrange("p (k c) -> p k c", c=FS),
                                 win[:, :].rearrange("(k p) c -> p k c", p=128))
                    lst.append(wsb)
                wsb = wp.tile([128, 3 * D], BF16, tag=f"wd{l}")
                q2.dma_start(wsb[:, 0:2 * D].rearrange("p (j c) -> p j c", c=D),
                             wd_in[l][0:256, :].rearrange("(j p) c -> p j c", p=128))
                q2.dma_start(wsb[0:96, 2 * D:3 * D], wd_in[l][256:FS, :])
                wd_sb.append(wsb)
            # lm_head interleaved with w_mean: sbuf col (k, j), j in [0, VS1)
            lmh_sb = wp.tile([128, KT * VS1], BF16, tag="lmh")
            lmh_v = lmh_sb[:].rearrange("p (k j) -> p k j", j=VS1)
            nc.gpsimd.dma_start(lmh_v[:, :, 0:VS],
                                lmh_in[:, :].rearrange("(k p) v -> p k v", p=128))
            nc.gpsimd.dma_start(lmh_v[:, :, VS:VS1],
                                wmean_in[:, :].rearrange("(k p) v -> p k v", p=128))

            # ---- KV caches ----
            kT_c = [kvp.tile([128, B * SMAX], BF16, tag=f"kT{l}", name=f"kT{l}")
                    for l in range(NL)]
            v0_c = [kvp.tile([128, 4 * HD], BF16, tag=f"v0{l}", name=f"v0{l}")
                    for l in range(NL)]
            v1_c = [kvp.tile([8, 4 * HD], BF16, tag=f"v1{l}", name=f"v1{l}")
                    for l in range(NL)]

            def rsqrt_lnexp(m_ap, shape, tag):
                # 1/sqrt(m) = exp(-0.5 ln m): two scalar ops, and Ln/Exp/Copy
                # live in one ACT table set so the scalar engine never swaps
                # tables anywhere in the kernel.
                tl = sb.tile(shape, F32, tag=f"{tag}b", name=f"{tag}b")
                nc.scalar.activation(tl[:], m_ap, AF.Ln)
                y = sb.tile(shape, F32, tag=f"{tag}t", name=f"{tag}t")
                nc.scalar.activation(y[:], tl[:], AF.Exp, scale=-0.5)
                return y[:]

            # ================= prefill =================
            hT = [hb.tile([128, PT], F32, tag=f"hT{k}", name=f"hT{k}", bufs=1)
                  for k in range(KT)]
            for k in range(KT):
                nc.sync.dma_start(hT[k][:], h0T[k * 128:(k + 1) * 128, :])

            def rms_norm_T(h_tiles):
                with psum_pool("pnorm", bufs=1) as pn:
                    ssp = pn.tile([1, PT], F32, tag="ssp", name="ssp")
                    for k in range(KT):
                        s = sb.tile([128, PT], F32, tag="sq", name="sq", bufs=1)
                        nc.vector.tensor_tensor(out=s[:], in0=h_tiles[k][:],
                                                in1=h_tiles[k][:], op=ALU.mult)
                        nc.tensor.matmul(out=ssp[:], lhsT=ones[:], rhs=s[:],
                                         start=(k == 0), stop=(k == KT - 1))
                    sd = sb.tile([1, PT], F32, tag="sd", name="sd")
                    nc.vector.tensor_scalar(out=sd[:], in0=ssp[:],
                                            scalar1=1.0 / D,
                                            scalar2=EPS,
                                            op0=ALU.mult, op1=ALU.add)
                r = rsqrt_lnexp(sd[:], [1, PT], "rpf")
                xb = []
                with psum_pool("prbc", bufs=1) as pb:
                    rbc = pb.tile([128, PT], F32, tag="rbc", name="rbc")
                    nc.tensor.matmul(out=rbc[:], lhsT=onesr[:], rhs=r,
                                     start=True, stop=True)
                    for k in range(KT):
                        x = sb.tile([128, PT], BF16, tag=f"xs{k}", name=f"xs{k}",
                                    bufs=1)
                        nc.vector.tensor_tensor(out=x[:], in0=h_tiles[k][:],
                                                in1=rbc[:], op=ALU.mult)
                        xb.append(x)
                return xb

            def ropeT(psrc, cache_dst=None):
                rot = sb.tile([128, PT], F32, tag="rot", name="rot", bufs=1)
                for h in range(HC):
                    b0 = h * HD
                    nc.scalar.activation(rot[b0:b0 + 32, :], psrc[b0 + 32:b0 + 64, :],
                                         AF.Copy)
                    nc.scalar.activation(rot[b0 + 32:b0 + 64, :], psrc[b0:b0 + 32, :],
                                         AF.Copy)
                t1 = sb.tile([128, PT], F32, tag="rt1", name="rt1", bufs=1)
                nc.vector.tensor_tensor(out=t1[:], in0=psrc[:], in1=pcosT[:],
                                        op=ALU.mult)
                nc.vector.tensor_tensor(out=rot[:], in0=rot[:], in1=psinT[:],
                                        op=ALU.mult)
                if cache_dst is None:
                    o = sb.tile([128, PT], BF16, tag="qro", name="qro", bufs=1)
                    nc.vector.tensor_tensor(out=o[:], in0=t1[:], in1=rot[:],
                                            op=ALU.add)
                    return o
                for b in range(B):
                    nc.vector.tensor_tensor(
                        out=cache_dst[:, b * SMAX:b * SMAX + L],
                        in0=t1[:, b * L:(b + 1) * L],
                        in1=rot[:, b * L:(b + 1) * L], op=ALU.add)
                return None

            def qkv_matmul(pool, xb, wsb, width, tag):
                ps = pool.tile([128, PT], F32, tag=tag, name=tag)
                for k in range(KT):
                    nc.tensor.matmul(out=ps[:width, :],
                                     lhsT=wsb[:, k * width:(k + 1) * width],
                                     rhs=xb[k][:], start=(k == 0), stop=(k == KT - 1))
                return ps

            def ar_big(psum_flat, h_tiles):
                bi = dp.tile([128, KT * PT], F32, tag="abi", name="abi")
                bo = dp.tile([128, KT * PT], F32, tag="abo", name="abo")
                for k in range(KT):
                    ev = sb.tile([128, PT], F32, tag="aev", name="aev", bufs=1)
                    nc.vector.tensor_copy(ev[:], psum_flat[:, k * PT:(k + 1) * PT])
                    nc.sync.dma_start(bi[:, k * PT:(k + 1) * PT], ev[:])
                nc.gpsimd.collective_compute("AllReduce", ALU.add, replica_groups=RG,
                                             ins=[bi[:].opt()], outs=[bo[:].opt()])
                for k in range(KT):
                    g = sb.tile([128, PT], F32, tag="aev", name="agt", bufs=1)
                    nc.sync.dma_start(g[:], bo[:, k * PT:(k + 1) * PT])
                    nc.vector.tensor_tensor(out=h_tiles[k][:], in0=h_tiles[k][:],
                                            in1=g[:], op=ALU.add)

            for l in range(NL):
                xb = rms_norm_T(hT)
                with psum_pool("pqkv", bufs=1) as pq_pool:
                    psq = qkv_matmul(pq_pool, xb, wqkv_sb[l][0], QC, "psq")
                    qb = ropeT(psq)
                    psk = qkv_matmul(pq_pool, xb, wqkv_sb[l][1], QC, "psk")
                    ropeT(psk, cache_dst=kT_c[l])
                    psv = qkv_matmul(pq_pool, xb, wqkv_sb[l][2], QC, "psv")
                    vb = sb.tile([128, PT], BF16, tag="vb", name="vb", bufs=1)
                    nc.scalar.activation(vb[:], psv[:], AF.Copy)
                with psum_pool("pvt", bufs=2) as pv_pool:
                    for b in range(B):
                        for h in range(HC):
                            p = 2 * b + h
                            pv = pv_pool.tile([128, HD], BF16, tag="pvT", name="pvT")
                            nc.tensor.transpose(
                                pv[:], vb[h * HD:(h + 1) * HD, b * L:(b + 1) * L],
                                idb[h * HD:(h + 1) * HD, h * HD:(h + 1) * HD])
                            nc.scalar.activation(v0_c[l][:, p * HD:(p + 1) * HD],
                                                 pv[:], AF.Copy)
                oT = sb.tile([128, PT], BF16, tag="oT", name="oT", bufs=1)
                with psum_pool("pattn", bufs=2) as pa_pool:
                    for b in range(B):
                        for h in range(HC):
                            p = 2 * b + h
                            psc = pa_pool.tile([L, L], F32, tag="psc", name="psc")
                            nc.tensor.matmul(
                                out=psc[:],
                                lhsT=qb[h * HD:(h + 1) * HD, b * L:(b + 1) * L],
                                rhs=kT_c[l][h * HD:(h + 1) * HD,
                                            b * SMAX:b * SMAX + L],
                                start=True, stop=True)
                            nc.vector.tensor_tensor(out=psc[:], in0=psc[:],
                                                    in1=cmask[:], op=ALU.add)
                            mx = sb.tile([L, 1], F32, tag="mx", name="mx")
                            nc.vector.tensor_reduce(mx[:], psc[:], AX.X, ALU.max)
                            nmx = sb.tile([L, 1], F32, tag="nmx", name="nmx")
                            nc.scalar.activation(nmx[:], mx[:], AF.Copy, scale=-0.125)
                            e = sb.tile([L, L], F32, tag="esm", name="esm", bufs=1)
                            nc.scalar.activation(e[:], psc[:], AF.Exp, bias=nmx[:],
                                                 scale=0.125)
                            ssum = sb.tile([L, 1], F32, tag="ssum", name="ssum")
                            nc.vector.tensor_reduce(ssum[:], e[:], AX.X, ALU.add)
                            rr = sb.tile([L, 1], F32, tag="rrp", name="rrp")
                            nc.vector.reciprocal(rr[:], ssum[:])
                            att = sb.tile([L, L], BF16, tag="att", name="att", bufs=1)
                            nc.vector.tensor_scalar_mul(att[:], e[:], rr[:])
                            paT = pa_pool.tile([L, L], BF16, tag="paT", name="paT")
                            nc.tensor.transpose(paT[:], att[:], idb[:L, :L])
                            attT = sb.tile([L, L], BF16, tag="attT", name="attT",
                                           bufs=1)
                            nc.scalar.activation(attT[:], paT[:], AF.Copy)
                            pov = pa_pool.tile([HD, L], F32, tag="pov", name="pov")
                            nc.tensor.matmul(out=pov[:],
                                             lhsT=v0_c[l][:, p * HD:(p + 1) * HD],
                                             rhs=attT[:], start=True, stop=True)
                            nc.scalar.activation(
                                oT[h * HD:(h + 1) * HD, b * L:(b + 1) * L],
                                pov[:], AF.Copy)
                with psum_pool("pbigp", bufs=1) as pb_pool:
                    pwo = pb_pool.tile([128, KT * PT], F32, tag="pbig", name="pwo")
                    for m in range(KT):
                        nc.tensor.matmul(out=pwo[:, m * PT:(m + 1) * PT],
                                         lhsT=wo_sb[l][:, m * 128:(m + 1) * 128],
                                         rhs=oT[:], start=True, stop=True)
                    ar_big(pwo, hT)
                # --- FFN ---
                xb2 = rms_norm_T(hT)
                af = sb.tile([128, 3 * PT], BF16, tag="af", name="af", bufs=1)
                with psum_pool("pffn", bufs=1) as pf_pool:
                    pg = pf_pool.tile([128, 3 * PT], F32, tag="pgu0", name="pg")
                    pu = pf_pool.tile([128, 3 * PT], F32, tag="pgu1", name="pu")
                    for ps, wsb in ((pg, wg_sb[l]), (pu, wu_sb[l])):
                        for j in range(3):
                            rows = min(128, FS - j * 128)
                            for k in range(KT):
                                nc.tensor.matmul(
                                    out=ps[:rows, j * PT:(j + 1) * PT],
                                    lhsT=wsb[:, k * FS + j * 128:
                                             k * FS + j * 128 + rows],
                                    rhs=xb2[k][:], start=(k == 0),
                                    stop=(k == KT - 1))
                    gs = sb.tile([128, 3 * PT], F32, tag="gsf", name="gsf", bufs=1)
                    for j in range(3):
                        rows = min(128, FS - j * 128)
                        blk = slice(j * PT, (j + 1) * PT)
                        # silu via exp only (scalar engine stays on one table)
                        nc.scalar.activation(gs[:rows, blk], pg[:rows, blk],
                                             AF.Exp, scale=-1.0)
                        nc.vector.tensor_scalar(out=gs[:rows, blk],
                                                in0=gs[:rows, blk], scalar1=1.0,
                                                scalar2=None, op0=ALU.add)
                        nc.vector.reciprocal(gs[:rows, blk], gs[:rows, blk])
                        nc.vector.tensor_tensor(out=gs[:rows, blk],
                                                in0=gs[:rows, blk],
                                                in1=pg[:rows, blk], op=ALU.mult)
                        nc.vector.tensor_tensor(out=af[:rows, blk],
                                                in0=gs[:rows, blk],
                                                in1=pu[:rows, blk], op=ALU.mult)
                with psum_pool("pbigp", bufs=1) as pb_pool:
                    pd = pb_pool.tile([128, KT * PT], F32, tag="pbig", name="pdd")
                    for m in range(KT):
                        for j in range(3):
                            rows = min(128, FS - j * 128)
                            nc.tensor.matmul(
                                out=pd[:, m * PT:(m + 1) * PT],
                                lhsT=wd_sb[l][:rows,
                                              j * D + m * 128:j * D + (m + 1) * 128],
                                rhs=af[:rows, j * PT:(j + 1) * PT],
                                start=(j == 0), stop=(j == 2))
                    ar_big(pd, hT)

            # last-token hidden -> row form [B, D]
            hrow0 = hb.tile([B, D], F32, tag="hrow", name="hrow0")
            with psum_pool("pxtp", bufs=2) as px_pool:
                for k in range(KT):
                    hl = sb.tile([128, B], F32, tag="hl", name="hl")
                    for b in range(B):
                        nc.scalar.activation(hl[:, b:b + 1],
                                             hT[k][:, b * L + L - 1:b * L + L],
                                             AF.Copy)
                    pt_ = px_pool.tile([B, 128], F32, tag="pxt", name="pxt")
                    nc.tensor.transpose(pt_[:], hl[:], idf[:, :])
                    nc.vector.tensor_copy(hrow0[:, k * 128:(k + 1) * 128], pt_[:])

            # ============ row-form helpers ============
            def rms_row(h):
                s = sb.tile([B, D], F32, tag="rs", name="rs", bufs=1)
                nc.vector.tensor_tensor(out=s[:], in0=h[:], in1=h[:], op=ALU.mult)
                ms = sb.tile([B, 1], F32, tag="rm", name="rm")
                nc.vector.tensor_reduce(ms[:], s[:], AX.X, ALU.add)
                sd = sb.tile([B, 1], F32, tag="rdd", name="rdd")
                nc.vector.tensor_scalar(out=sd[:], in0=ms[:], scalar1=1.0 / D,
                                        scalar2=EPS, op0=ALU.mult, op1=ALU.add)
                r = rsqrt_lnexp(sd[:], [B, 1], "rrow")
                x = sb.tile([B, D], BF16, tag="rx", name="rx")
                nc.vector.tensor_scalar_mul(x[:], h[:], r)
                return x

            def transpose_row(xrow, ncols, tag, in_f32=False):
                nt = (ncols + 127) // 128
                xT = sb.tile([128, 2 * nt], BF16, tag=tag, name=tag)
                with psum_pool("ptrp", bufs=2) as pr_pool:
                    for k in range(nt):
                        w = min(128, ncols - k * 128)
                        ptr = pr_pool.tile([128, B], F32 if in_f32 else BF16,
                                           tag="ptr", name="ptr")
                        ident = idf if in_f32 else idb
                        nc.tensor.transpose(ptr[:w, :], xrow[:, k * 128:k * 128 + w],
                                            ident[:B, :B])
                        nc.scalar.activation(xT[:w, 2 * k:2 * k + 2], ptr[:w, :],
                                             AF.Copy)
                return xT

            def ar_site(partial_sb, hin, tag, hname):
                # AllGather (floor ~4.6us vs AllReduce ~9.7us); the 16 gathered
                # rows land on 16 partitions and are summed per-b by a matmul
                # with the 0/1 selector selbd, fused with the residual add.
                bi = dp.tile([B, D], F32, tag=f"{tag}i", name=f"{tag}i")
                bo = dp.tile([TP * B, D], F32, tag=f"{tag}o", name=f"{tag}o")
                nc.sync.dma_start(bi[:], partial_sb[:])
                nc.gpsimd.collective_compute("AllGather", ALU.bypass,
                                             replica_groups=RG,
                                             ins=[bi[:].opt()], outs=[bo[:].opt()])
                g2 = sb.tile([TP * B, D], F32, tag="ag2", name=f"{tag}g2", bufs=1)
                nc.sync.dma_start(g2[:], bo[:])
                h2 = hb.tile([B, D], F32, tag="hrow", name=hname)
                with psum_pool("pags", bufs=1) as ps_pool:
                    pg2 = ps_pool.tile([B, D], F32, tag="pag", name="pag")
                    for half in range(2):
                        nc.tensor.matmul(out=pg2[:, half * 512:(half + 1) * 512],
                                         lhsT=selbd[:],
                                         rhs=g2[:, half * 512:(half + 1) * 512],
                                         start=True, stop=True)
                    nc.vector.tensor_tensor(out=h2[:], in0=hin[:], in1=pg2[:],
                                            op=ALU.add)
                return h2

            def vocab_step(tt, hrow):
                xf = rms_row(hrow)
                hfT = transpose_row(xf, D, "hfT")
                ll = sb.tile([B, VS], F32, tag="ll", name="ll", bufs=1)
                clrs = sb.tile([B, HOUT], F32, tag="dpart", name="clrs", bufs=1)
                m8x = sb.tile([B, 8 * NVC], F32, tag="m8x", name="m8x")
                i8x = sb.tile([B, 8 * NVC], U32, tag="i8x", name="i8x")
                meanv = sb.tile([B, 1], F32, tag="meanv", name="meanv")
                with psum_pool("plmp", bufs=4) as pl_pool:
                    for nt in range(NVC):
                        wid = VCW + 1 if nt == NVC - 1 else VCW
                        pl = pl_pool.tile([B, VCW + 1], F32, tag="plm", name="plm")
                        for k in range(KT):
                            nc.tensor.matmul(
                                out=pl[:, 0:wid], lhsT=hfT[:, 2 * k:2 * k + 2],
                                rhs=lmh_sb[:, k * VS1 + nt * VCW:
                                           k * VS1 + nt * VCW + wid],
                                start=(k == 0), stop=(k == KT - 1))
                        # copy on scalar; top-8 max+indices on vector — all
                        # overlap the remaining lm_head matmuls
                        nc.scalar.activation(ll[:, nt * VCW:(nt + 1) * VCW],
                                             pl[:, 0:VCW], AF.Copy)
                        nc.vector.max(m8x[:, nt * 8:(nt + 1) * 8],
                                      ll[:, nt * VCW:(nt + 1) * VCW])
                        nc.vector.max_index(i8x[:, nt * 8:(nt + 1) * 8],
                                            m8x[:, nt * 8:(nt + 1) * 8],
                                            ll[:, nt * VCW:(nt + 1) * VCW])
                        if nt == NVC - 1:
                            nc.scalar.activation(meanv[:], pl[:, VCW:VCW + 1],
                                                 AF.Copy, scale=1.0 / V)
                fmx = sb.tile([B, 1], F32, tag="fmx", name="fmx")
                nc.vector.tensor_reduce(fmx[:], m8x[:], AX.X, ALU.max)
                # global index of the max: indices+chunkbase where value==max,
                # min over candidates (ties -> lowest index, like argmax)
                gidxf = sb.tile([B, 8 * NVC], F32, tag="gidxf", name="gidxf")
                nc.vector.tensor_copy(gidxf[:], i8x[:])
                nc.vector.tensor_tensor(out=gidxf[:], in0=gidxf[:],
                                        in1=chunkbase[:], op=ALU.add)
                mkb = sb.tile([B, 8 * NVC], U32, tag="mkb", name="mkb")
                nc.vector.tensor_scalar(out=mkb[:], in0=m8x[:],
                                        scalar1=fmx[:, 0:1], scalar2=None,
                                        op0=ALU.is_equal)
                cand = sb.tile([B, 8 * NVC], F32, tag="cand", name="cand")
                nc.vector.select(cand[:], mkb[:], gidxf[:], big64[:])
                lvidx = sb.tile([B, 2], F32, tag="lvidx", name="lvidx")
                nc.vector.tensor_reduce(lvidx[:, 1:2], cand[:], AX.X, ALU.min)
                nc.vector.tensor_copy(lvidx[:, 0:1], fmx[:])
                # stats AllGather: (max, global_idx) per b -> [8, 4]
                sbi = dp.tile([1, 4], F32, tag="sti", name="sti")
                sbo = dp.tile([8, 4], F32, tag="sto", name="sto")
                nc.sync.dma_start(sbi[:], lvidx[:])
                nc.gpsimd.collective_compute("AllGather", ALU.bypass,
                                             replica_groups=RG,
                                             ins=[sbi[:].opt()], outs=[sbo[:].opt()])
                gsr = sb.tile([1, 32], F32, tag="gsr", name="gsr")
                nc.sync.dma_start(gsr[:], sbo[:])
                # gsr cols = (rank, b, j): j=0 max, j=1 idx
                gv = gsr[0:1, :].rearrange("p (r c) -> p c r", r=8)
                nv = sb.tile([1, 4], F32, tag="nv", name="nv")
                mk8 = sb.tile([1, 8], U32, tag="mk8", name="mk8")
                cand8 = sb.tile([1, 8], F32, tag="cand8", name="cand8")
                gmx = sb.tile([1, 1], F32, tag="gmx", name="gmx")
                for b in range(B):
                    mrow = gv[0:1, 2 * b:2 * b + 1, :]
                    irow = gv[0:1, 2 * b + 1:2 * b + 2, :]
                    nc.vector.tensor_reduce(gmx[:], mrow, AX.X, ALU.max)
                    nc.vector.tensor_scalar(out=mk8[:], in0=mrow, scalar1=gmx[:],
                                            scalar2=None, op0=ALU.is_equal)
                    nc.vector.select(cand8[:], mk8[:], irow, bigc[:])
                    nc.vector.tensor_reduce(nv[0:1, b:b + 1], cand8[:], AX.X,
                                            ALU.min)
                nc.vector.tensor_scalar(out=clrs[:], in0=ll[:, 0:HOUT],
                                        scalar1=meanv[:], scalar2=None,
                                        op0=ALU.subtract)
                nc.sync.dma_start(out_t[0:B, tt:tt + 1, :], clrs[:])
                if tt == T_NEW - 1:
                    return None
                with psum_pool("pgtp", bufs=1) as pg_pool:
                    pnid = pg_pool.tile([B, 1], F32, tag="pnid", name="pnid")
                    nc.tensor.transpose(pnid[:], nv[0:1, 0:2], idf[0:1, 0:1])
                    nidu = sb.tile([B, 1], U32, tag="nidu", name="nidu")
                    nc.vector.tensor_copy(nidu[:], pnid[:])
                embrow = hb.tile([B, D], F32, tag="hrow", name="embrow")
                nc.gpsimd.indirect_dma_start(
                    out=embrow[:], out_offset=None, in_=emb_in[:, :],
                    in_offset=bass.IndirectOffsetOnAxis(ap=nidu[:, 0:1], axis=0))
                return embrow

            # ============ decode ============
            def decode_layer(l, hin, t, dcst):
                slen = L + t
                x = rms_row(hin)
                xT = transpose_row(x, D, "xT")
                qk2 = sb.tile([B, 2 * QC], F32, tag="qk2", name="qk2", bufs=1)
                vrow = sb.tile([B, QC], BF16, tag="vrow", name="vrow", bufs=1)
                with psum_pool("pdq", bufs=1) as pq_pool:
                    pq = pq_pool.tile([B, 3 * QC], F32, tag="dqkv", name="dqkv")
                    for wi in range(3):
                        for k in range(KT):
                            nc.tensor.matmul(
                                out=pq[:, wi * QC:(wi + 1) * QC],
                                lhsT=xT[:, 2 * k:2 * k + 2],
                                rhs=wqkv_sb[l][wi][:, k * QC:(k + 1) * QC],
                                start=(k == 0), stop=(k == KT - 1))
                    rot = sb.tile([B, 2 * QC], F32, tag="rot", name="drot", bufs=1)
                    for blk in range(4):
                        c0 = blk * HD
                        nc.scalar.activation(rot[:, c0:c0 + 32],
                                             pq[:, c0 + 32:c0 + 64], AF.Copy)
                        nc.scalar.activation(rot[:, c0 + 32:c0 + 64],
                                             pq[:, c0:c0 + 32], AF.Copy)
                    t1 = sb.tile([B, 2 * QC], F32, tag="rt1", name="dt1", bufs=1)
                    nc.vector.tensor_tensor(
                        out=t1[:], in0=pq[:, 0:2 * QC],
                        in1=dcst[:, 0:4 * HD], op=ALU.mult)
                    nc.vector.tensor_tensor(
                        out=rot[:], in0=rot[:],
                        in1=dcst[:, 4 * HD:8 * HD], op=ALU.mult)
                    nc.vector.tensor_tensor(out=qk2[:], in0=t1[:], in1=rot[:],
                                            op=ALU.add)
                    nc.vector.tensor_copy(vrow[:], pq[:, 2 * QC:3 * QC])
                qkT = transpose_row(qk2, 2 * QC, "qkT", in_f32=True)
                for b in range(B):
                    nc.scalar.activation(
                        kT_c[l][:, b * SMAX + slen - 1:b * SMAX + slen],
                        qkT[:, 2 + b:3 + b], AF.Copy)
                    for h in range(HC):
                        p = 2 * b + h
                        nc.sync.dma_start(
                            v1_c[l][t - 1:t, p * HD:(p + 1) * HD],
                            vrow[b:b + 1, h * HD:(h + 1) * HD])
                ao = sb.tile([B, D], F32, tag="dpart", name="dao", bufs=1)
                with psum_pool("pdap", bufs=1) as pa_pool:
                    psc = pa_pool.tile([1, 4 * 512], F32, tag="dsc", name="dsc")
                    for b in range(B):
                        for h in range(HC):
                            p = 2 * b + h
                            nc.tensor.matmul(
                                out=psc[0:1, p * 512:p * 512 + slen],
                                lhsT=qkT[h * HD:(h + 1) * HD, b:b + 1],
                                rhs=kT_c[l][h * HD:(h + 1) * HD,
                                            b * SMAX:b * SMAX + slen],
                                start=True, stop=True)
                    mxr = sb.tile([1, 4], F32, tag="dmx", name="dmx")
                    for p in range(4):
                        nc.vector.tensor_reduce(
                            mxr[0:1, p:p + 1],
                            psc[0:1, p * 512:p * 512 + slen], AX.X, ALU.max)
                    nmx = sb.tile([1, 4], F32, tag="dnmx", name="dnmx")
                    nc.scalar.activation(nmx[:], mxr[:], AF.Copy, scale=-0.125)
                    er = sb.tile([1, 4 * SMAX], F32, tag="dscr", name="dscr", bufs=1)
                    for p in range(4):
                        nc.scalar.activation(
                            er[0:1, p * SMAX:p * SMAX + slen],
                            psc[0:1, p * 512:p * 512 + slen], AF.Exp,
                            bias=nmx[0:1, p:p + 1], scale=0.125)
                    ssum = sb.tile([1, 4], F32, tag="dss", name="dss")
                    for p in range(4):
                        nc.vector.tensor_reduce(
                            ssum[0:1, p:p + 1],
                            er[0:1, p * SMAX:p * SMAX + slen], AX.X, ALU.add)
                    rr = sb.tile([1, 4], F32, tag="drr", name="drr")
                    nc.vector.reciprocal(rr[:], ssum[:])
                    for p in range(4):
                        nc.vector.tensor_scalar_mul(
                            er[0:1, p * SMAX:p * SMAX + slen],
                            er[0:1, p * SMAX:p * SMAX + slen], rr[0:1, p:p + 1])
                    paT = pa_pool.tile([L, 4], F32, tag="dpaT", name="dpaT")
                    pa1 = pa_pool.tile([8, 4], F32, tag="dpa1", name="dpa1")
                    for p in range(4):
                        nc.tensor.transpose(paT[:, p:p + 1],
                                            er[0:1, p * SMAX:p * SMAX + L],
                                            idf[0:1, 0:1])
                        nc.tensor.transpose(pa1[0:t, p:p + 1],
                                            er[0:1, p * SMAX + L:p * SMAX + slen],
                                            idf[0:1, 0:1])
                    attT = sb.tile([L, 4], BF16, tag="dattT", name="dattT")
                    nc.scalar.activation(attT[:], paT[:], AF.Copy)
                    attT1 = sb.tile([8, 4], BF16, tag="dattT1", name="dattT1")
                    nc.scalar.activation(attT1[0:t, :], pa1[0:t, :], AF.Copy)
                    pov = pa_pool.tile([HD, 4], F32, tag="dpov", name="dpov")
                    for b in range(B):
                        for h in range(HC):
                            p = 2 * b + h
                            nc.tensor.matmul(out=pov[:, p:p + 1],
                                             lhsT=v0_c[l][:, p * HD:(p + 1) * HD],
                                             rhs=attT[:, p:p + 1], start=True,
                                             stop=False)
                            nc.tensor.matmul(out=pov[:, p:p + 1],
                                             lhsT=v1_c[l][0:t, p * HD:(p + 1) * HD],
                                             rhs=attT1[0:t, p:p + 1], start=False,
                                             stop=True)
                    doT = sb.tile([128, B], BF16, tag="doT", name="doT")
                    for b in range(B):
                        for h in range(HC):
                            p = 2 * b + h
                            nc.scalar.activation(doT[h * HD:(h + 1) * HD, b:b + 1],
                                                 pov[:, p:p + 1], AF.Copy)
                with psum_pool("pdwo", bufs=1) as pw_pool:
                    pwo = pw_pool.tile([B, D], F32, tag="dpwo", name="dpwo")
                    for half in range(2):
                        nc.tensor.matmul(
                            out=pwo[:, half * 512:(half + 1) * 512], lhsT=doT[:],
                            rhs=wo_sb[l][:, half * 512:(half + 1) * 512],
                            start=True, stop=True)
                    nc.vector.tensor_copy(ao[:], pwo[:])
                h2 = ar_site(ao, hin, "dA", "h2")
                # FFN
                x2 = rms_row(h2)
                x2T = transpose_row(x2, D, "x2T")
                aro = sb.tile([B, FS], F32, tag="daro", name="daro", bufs=1)
                with psum_pool("pdfp", bufs=1) as pf_pool:
                    pg = pf_pool.tile([B, 2 * 512], F32, tag="dpgu", name="dpgu")
                    for gi, wsb in enumerate((wg_sb[l], wu_sb[l])):
                        for k in range(KT):
                            nc.tensor.matmul(out=pg[:, gi * 512:gi * 512 + FS],
                                             lhsT=x2T[:, 2 * k:2 * k + 2],
                                             rhs=wsb[:, k * FS:(k + 1) * FS],
                                             start=(k == 0), stop=(k == KT - 1))
                    gsl = sb.tile([B, FS], F32, tag="dgs", name="dgs", bufs=1)
                    nc.scalar.activation(gsl[:], pg[:, 0:FS], AF.Exp, scale=-1.0)
                    nc.vector.tensor_scalar(out=gsl[:], in0=gsl[:], scalar1=1.0,
                                            scalar2=None, op0=ALU.add)
                    nc.vector.reciprocal(gsl[:], gsl[:])
                    nc.vector.tensor_tensor(out=gsl[:], in0=gsl[:],
                                            in1=pg[:, 0:FS], op=ALU.mult)
                    nc.vector.tensor_tensor(out=aro[:], in0=gsl[:],
                                            in1=pg[:, 512:512 + FS], op=ALU.mult)
                aT = transpose_row(aro, FS, "daT", in_f32=True)
                fo = sb.tile([B, D], F32, tag="dpart", name="dfo", bufs=1)
                with psum_pool("pddp", bufs=1) as pd_pool:
                    pd = pd_pool.tile([B, D], F32, tag="dpd", name="dpd")
                    for half in range(2):
                        for j in range(3):
                            rows = min(128, FS - j * 128)
                            nc.tensor.matmul(
                                out=pd[:, half * 512:(half + 1) * 512],
                                lhsT=aT[:rows, 2 * j:2 * j + 2],
                                rhs=wd_sb[l][:rows, j * D + half * 512:
                                             j * D + (half + 1) * 512],
                                start=(j == 0), stop=(j == 2))
                    nc.vector.tensor_copy(fo[:], pd[:])
                h3 = ar_site(fo, h2, "dF", "h3")
                return h3

            hcur = vocab_step(0, hrow0)
            for t in range(1, T_NEW):
                dcst = sb.tile([B, 8 * HD], F32, tag="dcst", name="dcst", bufs=2)
                nc.sync.dma_start(dcst[:], dcs_in[:, (t - 1) * 8 * HD:t * 8 * HD])
                for l in range(NL):
                    hcur = decode_layer(l, hcur, t, dcst)
                hcur = vocab_step(t, hcur)

    nc.compile()
    return nc


def make_in_maps(inputs):
    import ml_dtypes
    bf = ml_dtypes.bfloat16
    ii = {k: np.asarray(v) for k, v in inputs.items()}
    embed = ii["embed"].astype(np.float32)
    tokens = ii["input_ids"].astype(np.int64)
    h0 = embed[tokens]
    h0T = np.ascontiguousarray(h0.transpose(2, 0, 1).reshape(D, PT)).astype(np.float32)

    inv = ROPE_BASE ** (-np.arange(32, dtype=np.float64) / 32)
    dd = np.arange(HD)
    sgn = np.where(dd < 32, -1.0, 1.0)
    fr = inv[dd % 32]

    pos_p = np.tile(np.arange(L), B)
    ang_p = np.outer(fr, pos_p)
    pcosT = np.tile(np.cos(ang_p), (2, 1)).astype(np.float32)
    psinT = np.tile(sgn[:, None] * np.sin(ang_p), (2, 1)).astype(np.float32)

    pos_d = np.arange(L, L + T_NEW - 1)
    ang_d = np.outer(pos_d, fr)
    dcos = np.tile(np.cos(ang_d), (1, 4))                   # [T_NEW-1, 4*HD]
    dsin = np.tile(np.sin(ang_d) * sgn[None, :], (1, 4))
    dcs = np.tile(np.concatenate([dcos, dsin], axis=1).reshape(1, -1),
                  (B, 1)).astype(np.float32)

    q_idx = np.arange(L)[:, None]
    cmask = np.where(np.arange(L)[None, :] <= q_idx, 0.0, -8e9).astype(np.float32)
    idf = np.eye(128, dtype=np.float32)
    idb = np.eye(128).astype(bf)

    an, fn, fin = ii["attn_norm"], ii["ffn_norm"], ii["final_norm"]
    lmh_full = fin[:, None] * ii["lm_head"]
    wmean_full = np.mean(lmh_full, axis=1, keepdims=True)        # [D, 1]
    cb = np.repeat(np.arange(NVC, dtype=np.float32) * VCW, 8)    # [64]
    in_maps = []
    for c in range(TP):
        m = {"h0T": h0T, "emb": embed, "pcosT": pcosT, "psinT": psinT,
             "dcs": dcs, "cmask": cmask, "idf": idf,
             "idb": idb,
             "chunkbase": np.tile(cb + c * VS, (B, 1)).astype(np.float32),
             "wmean": wmean_full.astype(bf),
             "selbd": np.array([[1.0 if r % B == b else 0.0 for b in range(B)]
                                for r in range(TP * B)], np.float32)}
        for l in range(NL):
            for w, key in (("q", "wq"), ("k", "wk"), ("v", "wv")):
                m[f"w{w}{l}"] = np.ascontiguousarray(
                    (an[l][:, None] * ii[key][l])[:, c * QC:(c + 1) * QC]).astype(bf)
            m[f"wo{l}"] = np.ascontiguousarray(
                ii["wo"][l][c * QC:(c + 1) * QC, :]).astype(bf)
            m[f"wg{l}"] = np.ascontiguousarray(
                (fn[l][:, None] * ii["w_gate"][l])[:, c * FS:(c + 1) * FS]).astype(bf)
            m[f"wu{l}"] = np.ascontiguousarray(
                (fn[l][:, None] * ii["w_up"][l])[:, c * FS:(c + 1) * FS]).astype(bf)
            m[f"wd{l}"] = np.ascontiguousarray(
                ii["w_down"][l][c * FS:(c + 1) * FS, :]).astype(bf)
        m["lmh"] = np.ascontiguousarray(lmh_full[:, c * VS:(c + 1) * VS]).astype(bf)
        in_maps.append(m)
    return in_maps


_NC_CACHE = {}


def kernel(**inputs):
    if "nc" not in _NC_CACHE:
        _NC_CACHE["nc"] = build()
    nc = _NC_CACHE["nc"]
    in_maps = make_in_maps(inputs)
    res = bass_utils.run_bass_kernel_spmd(nc, in_maps, core_ids=list(range(TP)))
    return np.asarray(res.results[0]["out"], dtype=np.float32)

